# revision 1
# baseline (speedup 1.0000x reference)
"""Trainium2 Bass kernel for GCN-biased sparse attention (nn_Attention_37589553775245).

Reference computation (per batch b of 8, one NeuronCore each):
    qkv = x @ w_qkv; q,k,v per head (H=8, DH=64)
    attn = softmax(q k^T / sqrt(DH)) + A_hat        (A_hat = D^-1/2 (ceil(adj)+I) D^-1/2)
    out = (attn @ v) @ w_out + b_out

Sharding: pure batch-parallel across the 8 cores (B=8). A_hat is computed on
host (cheap) and replicated; weights replicated. No collectives.

Device-side layout strategy (all matmuls in float32r — tf32-class precision,
1 cycle/row at free-dim 512):
  - x is pre-transposed on host to xT [DIM, N] so the first matmul contraction
    (over DIM) sits on the partition axis.
  - q,k are produced transposed (qT,kT [DH, N]); v in natural [N, F] layout.
  - scores are computed transposed: sT[j,i] = sum_d k[j,d] qT[d,i], so the
    softmax denominator (sum over j) rides the attn@v matmul via an augmented
    V with a ones column: [v_h | 1] gives out rows 0..63 = (exp sT)^T v and
    row 64 = sum_j exp sT[j,i] (the denominator). Softmax max-subtraction is
    skipped: logits*scale ~ N(0,1), exp is safe in fp32.
  - adjacent heads (2h, 2h+1) sit at partition bases 0/64 of one qkT tile;
    q/k tiles are produced in head-pair order so early heads unblock first.
  - attention units are woven INTO phase 1 (head-0/1 q,k tiles + v first,
    remaining q/k tiles emitted between attention units) so the ACT engine's
    exp stream — the longest single-engine load, ~78 us — starts ~20 us
    earlier and the PE always has independent projection work while waiting;
    normalized attention outputs are written once into a dedicated yE tensor.
  - A_hat^T's 4MB SBUF residency is deferred: its tile allocates from the
    space freed by xT/w_qkv after phase 1, which is what lets the attention
    pools coexist with the phase-1 buffers under the 192KB/partition budget.
  - post-softmax bias contribution A_hat @ V is its own matmul (shared over
    heads), computed transposed via lhsT=V tiles, rhs=A_hat^T (host-shipped),
    woven between the late attention units (fully decoupled from the
    attention tails via yE) so the PE works while ACT drains the last exps.
  - Y^T = yE + (A_hat V)^T is merged per f-tile, then out = Y @ w_out + b_out
    with lhsT = Y^T tiles.
"""

import os
import sys

import numpy as np

for _p in ("/opt/trn_rl_repo", "/root/.axon_site/_ro/trn_rl_repo"):
    if _p not in sys.path and os.path.isdir(_p):
        sys.path.insert(0, _p)

import concourse.bass as bass  # noqa: E402
import concourse.mybir as mybir  # noqa: E402
import concourse.tile as tile  # noqa: E402
from concourse import bacc  # noqa: E402
from concourse.bass_utils import run_bass_kernel_spmd  # noqa: E402

B, N, DIM, H, DH = 8, 1024, 512, 8, 64
F = H * DH          # 512, inner dim
NT = N // 128       # 8 n-tiles (also j-tiles)
DT = DIM // 128     # 4 dim-tiles
FT = F // 128       # 4 f-tiles
NC2 = N // 512      # 2 i-chunks of 512
SCALE = DH ** -0.5

F32 = mybir.dt.float32
F32R = mybir.dt.float32r

_PROGRAM = None
_last_in_maps = None


def _build_program(reps=1, copies_dve=True, exp_batch=2, interleave=False,
                   pair_heads=False, skip_exp=False, skip_attn=False,
                   mm_bufs=2, s_bufs=None, o_bufs=None, exps_bufs=6,
                   early_attn=True):
    nc = bacc.Bacc("TRN2", target_bir_lowering=False, debug=False, num_devices=8)

    xT_d = nc.dram_tensor("xT", [DIM, N], F32R, kind="ExternalInput")
    wqkv_d = nc.dram_tensor("wqkv", [DIM, 3 * F], F32R, kind="ExternalInput")
    ahatT_d = nc.dram_tensor("ahatT", [N, N], F32R, kind="ExternalInput")
    wout_d = nc.dram_tensor("wout", [F, DIM], F32R, kind="ExternalInput")
    bout_d = nc.dram_tensor("bout", [1, DIM], F32, kind="ExternalInput")
    out_d = nc.dram_tensor("out", [N, DIM], F32, kind="ExternalOutput")

    def copy_out(dst, src):
        if copies_dve:
            nc.vector.tensor_copy(out=dst, in_=src)
        else:
            nc.scalar.copy(out=dst, in_=src)

    # PSUM budget is 8 banks total; a [128, 512] fp32 tile is one bank.
    if pair_heads:
        exp_batch = 1       # score tiles stay 1 bank; 2 in flight per jb
        s_bufs = 3 if s_bufs is None else s_bufs
        o_bufs = 3 if o_bufs is None else o_bufs
    else:
        s_bufs = 2 if s_bufs is None else s_bufs
        o_bufs = 2 if o_bufs is None else o_bufs
    with tile.TileContext(nc) as tc:
        with (
            tc.tile_pool(name="big", bufs=1) as big,
            tc.tile_pool(name="ps_mm", bufs=mm_bufs, space="PSUM") as ps_mm,
            tc.tile_pool(name="ps_s", bufs=s_bufs, space="PSUM") as ps_s,
            tc.tile_pool(name="ps_o", bufs=o_bufs, space="PSUM") as ps_o,
        ):
          for _rep in range(reps):
            # ---- persistent SBUF tensors -------------------------------
            # (ahatT is allocated later, reusing phase-1's freed SBUF)
            wout = big.tile([128, FT, DIM], F32R)
            qkT = big.tile([128, 2 * FT, N], F32R)       # [f, n] f=q(0:512),k(512:1024)
            v_sb = big.tile([128, NT, F], F32R)          # v[n, f]
            vaug = big.tile([128, NT, H, DH + 1], F32R)  # [n, h, v|1]
            yT = big.tile([128, FT, N], F32R)            # Y^T[f, i] (A_hat V part)
            yE = big.tile([128, FT, N], F32R)            # Y^T (exp-attention part)
            bout_bc = big.tile([128, DIM], F32)

            nc.vector.memset(vaug.bitcast(F32), 1.0)  # ones col survives v copies

            # attention-unit pools open BEFORE phase 1 so attention can be
            # emitted interleaved with the projections (fits SBUF because
            # ahatT's 32KB/partition is deferred past the ph1 release)
            exps = tc.alloc_tile_pool(name="exps", bufs=exps_bufs)
            small = tc.alloc_tile_pool(name="small", bufs=2)
            outs = tc.alloc_tile_pool(name="outs", bufs=3)
            dscr = tc.alloc_tile_pool(name="dscr", bufs=4, space="DRAM")

            # ---- phase 1: qT/kT (transposed) and v (natural) -----------
            ph1 = tc.alloc_tile_pool(name="ph1", bufs=1)
            if True:
                xT = ph1.tile([128, DT, N], F32R)        # xT[dim, n]
                wqkv = ph1.tile([128, DT, 3 * F], F32R)
                # phase-1-gating loads go first on the sync HWDGE ring (FIFO);
                # the big A_hat^T load rides the idle SWDGE ring in parallel.
                nc.sync.dma_start(
                    out=xT,
                    in_=xT_d[:, :].rearrange("(t p) n -> p t n", p=128),
                )
                for fc in range(6):   # 256-col chunks so early groups unblock
                    nc.sync.dma_start(
                        out=wqkv[:, :, fc * 256:(fc + 1) * 256],
                        in_=wqkv_d[:, fc * 256:(fc + 1) * 256].rearrange(
                            "(t p) f -> p t f", p=128),
                    )
                nc.sync.dma_start(
                    out=wout,
                    in_=wout_d[:, :].rearrange("(t p) n -> p t n", p=128),
                )
                nc.sync.dma_start(out=bout_bc, in_=bout_d[0:1, :].to_broadcast((128, DIM)))
            def emit_qk(ft):
                for c in range(NC2):
                    ps = ps_mm.tile([128, 512], F32, tag="mm")
                    for dt_i in range(DT):
                        nc.tensor.matmul(
                            ps,
                            wqkv[:, dt_i, ft * 128:(ft + 1) * 128],
                            xT[:, dt_i, c * 512:(c + 1) * 512],
                            start=(dt_i == 0),
                            stop=(dt_i == DT - 1),
                        )
                    copy_out(qkT[:, ft, c * 512:(c + 1) * 512], ps)

            def emit_v():
                for nt in range(NT):
                    ps = ps_mm.tile([128, 512], F32, tag="mm")
                    for dt_i in range(DT):
                        nc.tensor.matmul(
                            ps,
                            xT[:, dt_i, nt * 128:(nt + 1) * 128],
                            wqkv[:, dt_i, 2 * F:3 * F],
                            start=(dt_i == 0),
                            stop=(dt_i == DT - 1),
                        )
                    copy_out(v_sb[:, nt, :], ps)
                    nc.vector.tensor_copy(
                        out=vaug[:, nt, :, 0:DH],
                        in_=ps.rearrange("p (h d) -> p h d", h=H),
                    )

            def ahat_unit(ft, c):
                # (A_hat @ V)^T [f-tile ft, i-chunk c] -> yT
                ps = ps_mm.tile([128, 512], F32, tag="mm")
                for jt in range(NT):
                    nc.tensor.matmul(
                        ps,
                        v_sb[:, jt, ft * 128:(ft + 1) * 128],
                        ahatT[:, jt, c * 512:(c + 1) * 512],
                        start=(jt == 0),
                        stop=(jt == NT - 1),
                    )
                copy_out(yT[:, ft, c * 512:(c + 1) * 512], ps)

            def attn_tail(h, c, ps_out):
                # normalize expv^T by the ridden denominator; write-once into
                # yE (merged with the A_hat V part in yT before projection)
                hb = (h % 2) * 64
                recip = small.tile([65, 512], F32, tag="recip")
                nc.vector.reciprocal(out=recip[64:65, :], in_=ps_out[64:65, :])
                # partition-broadcast via DRAM bounce (SBUF sources can't
                # broadcast across partitions; DRAM sources can)
                scr = dscr.tile([1, 512], F32, tag="scr")
                nc.sync.dma_start(out=scr, in_=recip[64:65, :])
                bcast = small.tile([64, 512], F32, tag="bcast")
                nc.sync.dma_start(out=bcast, in_=scr.to_broadcast((64, 512)))
                ysl = yE[hb:hb + 64, h // 2, c * 512:(c + 1) * 512]
                if hb == 0:
                    nc.vector.tensor_mul(ysl, ps_out[0:64, :], bcast)
                else:
                    # DVE lanes can't shift partitions; write at base 0 and
                    # SWDGE-copy into the base-64 yE slice.
                    prod = small.tile([64, 512], F32R, tag="prod")
                    nc.vector.tensor_mul(prod, ps_out[0:64, :], bcast)
                    nc.gpsimd.dma_start(out=ysl, in_=prod)

            def attn_unit(h, c):
                # one head, one 512-wide i-chunk
                hb = (h % 2) * 64
                ht = h // 2
                ps_out = ps_o.tile([65, 512], F32, tag="po")
                for jb in range(NT // exp_batch):
                    ps_sc = ps_s.tile([128, exp_batch, 512], F32, tag="ps")
                    for e in range(exp_batch):
                        jt = jb * exp_batch + e
                        # scoresT[j, i] = sum_d kT[d, j] qT[d, i]
                        nc.tensor.matmul(
                            ps_sc[:, e, :],
                            qkT[hb:hb + 64, FT + ht, jt * 128:(jt + 1) * 128],
                            qkT[hb:hb + 64, ht, c * 512:(c + 1) * 512],
                        )
                    et = exps.tile([128, exp_batch, 512], F32R, tag="exp")
                    if skip_exp:
                        nc.vector.tensor_copy(out=et, in_=ps_sc)
                    else:
                        nc.scalar.activation(
                            out=et, in_=ps_sc,
                            func=mybir.ActivationFunctionType.Exp,
                            scale=float(SCALE),
                        )
                    for e in range(exp_batch):
                        jt = jb * exp_batch + e
                        # [expv^T ; denom] accumulation
                        nc.tensor.matmul(
                            ps_out,
                            vaug[:, jt, h, :],
                            et[:, e, :],
                            start=(jt == 0),
                            stop=(jt == NT - 1),
                        )
                attn_tail(h, c, ps_out)

            def attn_unit_pair(hp, c):
                # heads 2hp (rows 0:64) and 2hp+1 (rows 64:128) share qkT
                # tiles; their score matmuls hit disjoint PE row groups and
                # run concurrently.
                ht = hp
                po = [ps_o.tile([65, 512], F32, tag="po", name=f"po{u}")
                      for u in range(2)]
                for jb in range(NT // exp_batch):
                    ps_sc = [ps_s.tile([128, exp_batch, 512], F32, tag="ps",
                                       name=f"ps_sc{u}")
                             for u in range(2)]
                    for e in range(exp_batch):
                        jt = jb * exp_batch + e
                        for u, hb in enumerate((0, 64)):
                            nc.tensor.matmul(
                                ps_sc[u][:, e, :],
                                qkT[hb:hb + 64, FT + ht, jt * 128:(jt + 1) * 128],
                                qkT[hb:hb + 64, ht, c * 512:(c + 1) * 512],
                            )
                    ets = []
                    for u in range(2):
                        et = exps.tile([128, exp_batch, 512], F32R, tag="exp",
                                       name=f"et{u}")
                        if skip_exp:
                            nc.vector.tensor_copy(out=et, in_=ps_sc[u])
                        else:
                            nc.scalar.activation(
                                out=et, in_=ps_sc[u],
                                func=mybir.ActivationFunctionType.Exp,
                                scale=float(SCALE),
                            )
                        ets.append(et)
                    for e in range(exp_batch):
                        jt = jb * exp_batch + e
                        for u in range(2):
                            nc.tensor.matmul(
                                po[u],
                                vaug[:, jt, 2 * hp + u, :],
                                ets[u][:, e, :],
                                start=(jt == 0),
                                stop=(jt == NT - 1),
                            )
                for u in range(2):
                    attn_tail(2 * hp + u, c, po[u])

            # ---- emission: phase 1 woven with attention ----------------
            # head-0/1 q,k tiles + all of v first, then attention units with
            # the remaining q/k tiles woven between them (PE gets independent
            # work while ACT chews the exp stream, which starts ~20us sooner).
            emit_qk(0)
            emit_qk(4)
            emit_v()
            rest_qk = [1, 5, 2, 6, 3, 7]
            ahat_left = [(ft, c) for ft in range(FT) for c in range(NC2)]
            ahatT = None

            def emit_ahat():
                ft, c = ahat_left.pop(0)
                ahat_unit(ft, c)
                if c == NC2 - 1 and not skip_attn:
                    # merge Y^T parts per f-tile; DVE overlaps later A_hat MMs
                    nc.vector.tensor_add(yT[:, ft, :], yT[:, ft, :],
                                         yE[:, ft, :])

            if skip_attn:
                for ft in rest_qk:
                    emit_qk(ft)
                ph1.release()
                ahatp = tc.alloc_tile_pool(name="ahatp", bufs=1)
                ahatT = ahatp.tile([128, NT, N], F32R)
                nc.sync.dma_start(
                    out=ahatT,
                    in_=ahatT_d[:, :].rearrange("(t p) n -> p t n", p=128),
                )
                while ahat_left:
                    emit_ahat()
            else:
                qi = 0
                for u, (h, c) in enumerate([(h, c) for h in range(H)
                                            for c in range(NC2)]):
                    attn_unit(h, c)
                    if qi < len(rest_qk):
                        emit_qk(rest_qk[qi])
                        qi += 1
                        if qi == len(rest_qk):
                            # phase-1 buffers die here; A_hat^T lands in the
                            # freed SBUF while attention continues
                            ph1.release()
                            ahatp = tc.alloc_tile_pool(name="ahatp", bufs=1)
                            ahatT = ahatp.tile([128, NT, N], F32R)
                            nc.sync.dma_start(
                                out=ahatT,
                                in_=ahatT_d[:, :].rearrange(
                                    "(t p) n -> p t n", p=128),
                            )
                    elif u >= 10:
                        # weave A_hat units between late attention units so
                        # the PE works while ACT drains the last exp batches
                        emit_ahat()
                while ahat_left:
                    emit_ahat()

            # ---- phase 4: out = Y @ w_out + b_out ----------------------
            for nt in range(NT):
                ps = ps_mm.tile([128, 512], F32, tag="mm")
                for ft in range(FT):
                    nc.tensor.matmul(
                        ps,
                        yT[:, ft, nt * 128:(nt + 1) * 128],
                        wout[:, ft, :],
                        start=(ft == 0),
                        stop=(ft == FT - 1),
                    )
                ot = outs.tile([128, DIM], F32, tag="ot")
                nc.vector.tensor_add(ot, ps, bout_bc)
                nc.sync.dma_start(out=out_d[nt * 128:(nt + 1) * 128, :], in_=ot)

            ahatp.release()
            dscr.release()
            outs.release()
            small.release()
            exps.release()

    nc.compile()
    return nc


def _get_program():
    global _PROGRAM
    if _PROGRAM is None:
        _PROGRAM = _build_program()
    return _PROGRAM


def kernel(x, adj, w_qkv, w_out, b_out):
    x = np.asarray(x, dtype=np.float32)
    adj = np.asarray(adj, dtype=np.float32)
    w_qkv = np.ascontiguousarray(np.asarray(w_qkv, dtype=np.float32))
    w_out = np.ascontiguousarray(np.asarray(w_out, dtype=np.float32))
    b_out = np.asarray(b_out, dtype=np.float32).reshape(1, DIM)

    # host-side: normalized adjacency bias, replicated (cheap: one 1024^2 pass)
    A = np.ceil(adj) + np.eye(N, dtype=np.float32)
    dinv = A.sum(axis=1) ** -0.5
    A_hat = (A * dinv[:, None]) * dinv[None, :]
    ahatT = np.ascontiguousarray(A_hat.T)

    nc = _get_program()
    in_maps = []
    for b in range(B):
        in_maps.append({
            "xT": np.ascontiguousarray(x[b].T),
            "wqkv": w_qkv,
            "ahatT": ahatT,
            "wout": w_out,
            "bout": b_out,
        })
    global _last_in_maps
    _last_in_maps = in_maps
    res = run_bass_kernel_spmd(nc, in_maps, list(range(B)))
    out = np.stack([res.results[b]["out"] for b in range(B)], axis=0)
    return out.astype(np.float32)


if __name__ == "__main__":
    rng = np.random.default_rng(0)
    x = rng.standard_normal((B, N, DIM), dtype=np.float32)
    adj = (rng.random((N, N), dtype=np.float32) < 0.05).astype(np.float32) * 0.5
    w_qkv = rng.standard_normal((DIM, 3 * F), dtype=np.float32) * DIM ** -0.5
    w_out = rng.standard_normal((F, DIM), dtype=np.float32) * F ** -0.5
    b_out = np.zeros(DIM, dtype=np.float32)
    out = kernel(x=x, adj=adj, w_qkv=w_qkv, w_out=w_out, b_out=b_out)
    print("out", out.shape, out.dtype, np.abs(out).max())



# revision 18
# speedup vs baseline: 6.5588x; 6.5588x over previous
"""Trainium2 Bass kernel for GCN-biased sparse attention (nn_Attention_37589553775245).

Reference computation (per batch b of 8, one NeuronCore each):
    qkv = x @ w_qkv; q,k,v per head (H=8, DH=64)
    attn = softmax(q k^T / sqrt(DH)) + A_hat        (A_hat = D^-1/2 (ceil(adj)+I) D^-1/2)
    out = (attn @ v) @ w_out + b_out

Sharding: pure batch-parallel across the 8 cores (B=8). A_hat is computed on
host (cheap) and replicated; weights replicated. No collectives.

v2 design notes (vs the fp32r v1 baseline):
  - ALL matmul operands are bf16 (x, w_qkv, w_out, A_hat^T, qkT, v, exp(s)):
    same PE cost (1 cycle/row) but half the DMA bytes and SBUF residency;
    fp32 PSUM accumulation keeps rel-err ~1e-3 (tolerance 2e-2).
  - scores are computed transposed (sT[j,i]) so softmax denominators ride the
    attn@v matmul via ones columns in an augmented V laid out [1 | v | 1]:
    even heads use cols 1:66 (denom = out row 64, partition base 0), odd heads
    use cols 0:65 with PSUM out at partition base 63 (denom row 63, v rows
    64:128) — output lands directly at yE partition base 64, removing the
    partition-shift SWDGE copy the v1 kernel needed for odd heads.
  - unit order is i-chunk-major: all 8 heads of chunk 0, then chunk 1. Chunk-0
    merges + out-projections for rows 0:512 run woven into chunk-1 attention,
    shortening the serial tail.
  - loads are chunked and split across the three DMA queues (SP ring: xT
    d-chunks + bias; ACT ring: w_qkv chunks + w_out; SWDGE: A_hat^T) so the
    first projection matmul is gated by ~2 small transfers, not one 2MB load.
  - PSUM->SBUF copies for qkT/v/yT go on the (otherwise idle) Pool/GPSIMD
    engine; attention-tail outputs are Pool-copied out of PSUM immediately so
    score/output banks recycle without waiting on the reciprocal-broadcast
    DRAM bounce (partition broadcast needs a DRAM-sourced DMA).
  - everything is SBUF-resident simultaneously (bf16 halves footprints), so
    A_hat^T loads up front on the SWDGE ring and A_hat@V units weave into the
    chunk-0 attention stream as early PE filler.
"""

import os
import sys

import numpy as np

for _p in ("/opt/trn_rl_repo", "/root/.axon_site/_ro/trn_rl_repo"):
    if _p not in sys.path and os.path.isdir(_p):
        sys.path.insert(0, _p)

import ml_dtypes  # noqa: E402

import concourse.bass as bass  # noqa: E402
import concourse.mybir as mybir  # noqa: E402
import concourse.tile as tile  # noqa: E402
from concourse import bacc  # noqa: E402
from concourse.bass_utils import run_bass_kernel_spmd  # noqa: E402

B, N, DIM, H, DH = 8, 1024, 512, 8, 64
F = H * DH          # 512, inner dim
NT = N // 128       # 8 n-tiles (also j-tiles)
DT = DIM // 128     # 4 dim-tiles
FT = F // 128       # 4 f-tiles
NC2 = N // 512      # 2 i-chunks of 512
SCALE = DH ** -0.5

F32 = mybir.dt.float32
BF16 = mybir.dt.bfloat16

_PROGRAM = None
_last_in_maps = None


def _build_program(reps=1, qk_copies_pool=True, o_copy_pool=True,
                   s_bufs=2, o_bufs=2, mm_bufs=2, exps_bufs=6,
                   mmdt="bf16", per2_bufs=2):
    MDT = {"bf16": BF16, "f32r": mybir.dt.float32r}[mmdt]
    nc = bacc.Bacc("TRN2", target_bir_lowering=False, debug=False, num_devices=8)

    xT_d = nc.dram_tensor("xT", [DIM, N], MDT, kind="ExternalInput")
    wqkv_d = nc.dram_tensor("wqkv", [DIM, 3 * F], MDT, kind="ExternalInput")
    ahatT_d = nc.dram_tensor("ahatT", [N, N], MDT, kind="ExternalInput")
    wout_d = nc.dram_tensor("wout", [F, DIM], MDT, kind="ExternalInput")
    bout_d = nc.dram_tensor("bout", [1, DIM], F32, kind="ExternalInput")
    out_d = nc.dram_tensor("out", [N, DIM], F32, kind="ExternalOutput")

    with tile.TileContext(nc) as tc:
        with (
            nc.allow_low_precision(
                reason="bf16 softmax-normalize/merge; fp32 PSUM accumulation "
                       "everywhere it matters, tolerance is 2e-2"),
            tc.tile_pool(name="per2", bufs=per2_bufs) as per2,
            tc.tile_pool(name="per1", bufs=1) as per1,
            tc.tile_pool(name="exps", bufs=exps_bufs) as exps,
            tc.tile_pool(name="small", bufs=3) as small,
            tc.tile_pool(name="outs", bufs=3) as outs,
            tc.tile_pool(name="dscr", bufs=4, space="DRAM") as dscr,
            tc.tile_pool(name="ps_mm", bufs=mm_bufs, space="PSUM") as ps_mm,
            tc.tile_pool(name="ps_s", bufs=s_bufs, space="PSUM") as ps_s,
            tc.tile_pool(name="ps_o", bufs=o_bufs, space="PSUM") as ps_o,
        ):
          for _rep in range(reps):
            # ---- persistent SBUF tensors -------------------------------
            # per2 (double-buffered): early-lifetime tensors, so rep r+1's
            # projections can overlap rep r's attention when reps>1.
            xT = per2.tile([128, DT, N], MDT)           # xT[dim, n]
            wqkv = per2.tile([128, DT, 3 * F], MDT)
            qkT = per2.tile([128, 2 * FT, N], MDT)      # [f, n] f=q(0:512),k(512:1024)
            v_sb = per2.tile([128, NT, F], MDT)         # v[n, f]
            vaug = per2.tile([128, NT, H, DH + 1], MDT)  # [n, h, v|1]
            wout = per1.tile([128, FT, DIM], MDT)
            ahatT = per1.tile([128, NT, N], MDT)
            yT = per1.tile([128, FT, N], MDT)           # Y^T[f, i] (A_hat V part)
            yE = per1.tile([128, FT, N], MDT)           # Y^T (exp-attention part)
            bout_bc = per1.tile([128, DIM], F32)

            # only the ones-column needs initializing (v copies fill the rest)
            nc.gpsimd.memset(vaug[:, :, :, DH:DH + 1], 1.0)

            # ---- loads: split across the three DMA queues, ordered so the
            # emit_qk(0)/emit_qk(4) gating chunks land first ---------------
            def load_wqkv(fc, eng):
                eng.dma_start(
                    out=wqkv[:, :, fc * 256:(fc + 1) * 256],
                    in_=wqkv_d[:, fc * 256:(fc + 1) * 256].rearrange(
                        "(t p) f -> p t f", p=128),
                )

            def load_xt(dt_i):
                nc.sync.dma_start(
                    out=xT[:, dt_i, :],
                    in_=xT_d[dt_i * 128:(dt_i + 1) * 128, :],
                )

            load_xt(0)
            load_wqkv(0, nc.sync)        # q cols 0:256 (heads 0..3)
            load_wqkv(2, nc.sync)        # k cols 512:768 (heads 0..3)
            for dt_i in range(1, DT):
                load_xt(dt_i)
            nc.sync.dma_start(out=bout_bc, in_=bout_d[0:1, :].to_broadcast((128, DIM)))
            for fc in (4, 5, 1, 3):      # ACT ring: v cols first, rest of q/k
                load_wqkv(fc, nc.scalar)
            nc.scalar.dma_start(
                out=wout,
                in_=wout_d[:, :].rearrange("(t p) n -> p t n", p=128),
            )
            # A_hat^T rides the ACT HWDGE ring last: off the critical path,
            # and NOT on the SWDGE/Pool queue (a software-driven SWDGE DMA
            # occupies the Pool sequencer for its whole transfer, which would
            # block the qkT/v PSUM->SBUF Pool copies behind it).
            nc.scalar.dma_start(
                out=ahatT,
                in_=ahatT_d[:, :].rearrange("(t p) n -> p t n", p=128),
            )

            # PSUM->SBUF copies: GPSIMD/Pool cannot access PSUM on real
            # TRN2 (BIR verifier rejects it), so spread them over ACT (idle
            # during the projection phase) and DVE.
            def qk_copy(dst, src):
                if qk_copies_pool:
                    nc.scalar.copy(out=dst, in_=src)
                else:
                    nc.vector.tensor_copy(out=dst, in_=src)

            def dve_copy(dst, src):
                nc.vector.tensor_copy(out=dst, in_=src)

            # ---- phase 1: qT/kT (transposed) and v (natural) -----------
            def emit_qk(ft):
                for c in range(NC2):
                    ps = ps_mm.tile([128, 512], F32, tag="mm")
                    for dt_i in range(DT):
                        nc.tensor.matmul(
                            ps,
                            wqkv[:, dt_i, ft * 128:(ft + 1) * 128],
                            xT[:, dt_i, c * 512:(c + 1) * 512],
                            start=(dt_i == 0),
                            stop=(dt_i == DT - 1),
                        )
                    qk_copy(qkT[:, ft, c * 512:(c + 1) * 512], ps)

            def emit_v(nt_lo=0, nt_hi=NT):
                for nt in range(nt_lo, nt_hi):
                    ps = ps_mm.tile([128, 512], F32, tag="mm")
                    for dt_i in range(DT):
                        nc.tensor.matmul(
                            ps,
                            xT[:, dt_i, nt * 128:(nt + 1) * 128],
                            wqkv[:, dt_i, 2 * F:3 * F],
                            start=(dt_i == 0),
                            stop=(dt_i == DT - 1),
                        )
                    dve_copy(v_sb[:, nt, :], ps)
                    nc.vector.tensor_copy(
                        out=vaug[:, nt, :, 0:DH],
                        in_=ps.rearrange("p (h d) -> p h d", h=H),
                    )

            def ahat_unit(ft, c):
                # (A_hat @ V)^T [f-tile ft, i-chunk c] -> yT
                ps = ps_mm.tile([128, 512], F32, tag="mm")
                for jt in range(NT):
                    nc.tensor.matmul(
                        ps,
                        v_sb[:, jt, ft * 128:(ft + 1) * 128],
                        ahatT[:, jt, c * 512:(c + 1) * 512],
                        start=(jt == 0),
                        stop=(jt == NT - 1),
                    )
                dve_copy(yT[:, ft, c * 512:(c + 1) * 512], ps)

            def attn_tail(h, c, po):
                # normalize exp-attn rows by the ridden denominator, write
                # once into yE. PSUM matmul outputs must sit at partition
                # base 0/32/64, so both parities compute at rows 0:65; odd
                # heads SWDGE-shift the bf16 rows to partitions 64:128
                # (the shift hides under the broadcast DRAM bounce).
                odd = h % 2 == 1
                # Pool-copy out of PSUM so the bank recycles without waiting
                # on the broadcast bounce; bf16 from here on.
                osb = small.tile([128, 512], BF16, tag="osb")
                nc.vector.tensor_copy(out=osb[0:65, :], in_=po[0:65, :])
                recip = small.tile([128, 512], BF16, tag="recip")
                nc.vector.reciprocal(out=recip[64:65, :], in_=osb[64:65, :])
                # partition-broadcast via DRAM bounce (SBUF sources can't
                # broadcast across partitions; DRAM sources can), landing at
                # the same partition range as the final yE slice.
                scr = dscr.tile([1, 512], BF16, tag="scr")
                nc.sync.dma_start(out=scr, in_=recip[64:65, :])
                vlo, vhi = (64, 128) if odd else (0, 64)
                bcast = small.tile([128, 512], BF16, tag="bcast")
                nc.sync.dma_start(out=bcast[vlo:vhi, :],
                                  in_=scr.to_broadcast((64, 512)))
                if odd:
                    nc.gpsimd.dma_start(out=osb[64:128, :], in_=osb[0:64, :])
                ysl = yE[vlo:vhi, h // 2, c * 512:(c + 1) * 512]
                nc.vector.tensor_mul(ysl, osb[vlo:vhi, :], bcast[vlo:vhi, :])

            def scores_jb(h, c, jb):
                # one 2-j-tile score block + its exp; returns the et tile
                hb = (h % 2) * 64
                ht = h // 2
                ps_sc = ps_s.tile([128, 2, 512], F32, tag="ps")
                for e in range(2):
                    jt = jb * 2 + e
                    # scoresT[j, i] = sum_d kT[d, j] qT[d, i]
                    nc.tensor.matmul(
                        ps_sc[:, e, :],
                        qkT[hb:hb + 64, FT + ht, jt * 128:(jt + 1) * 128],
                        qkT[hb:hb + 64, ht, c * 512:(c + 1) * 512],
                    )
                et = exps.tile([128, 2, 512], MDT, tag="exp")
                nc.scalar.activation(
                    out=et, in_=ps_sc,
                    func=mybir.ActivationFunctionType.Exp,
                    scale=float(SCALE),
                )
                return et

            def av_jb(h, out_ap, jb, et):
                for e in range(2):
                    jt = jb * 2 + e
                    # [expv^T ; denom] accumulation
                    nc.tensor.matmul(
                        out_ap,
                        vaug[:, jt, h, :],
                        et[:, e, :],
                        start=(jt == 0),
                        stop=(jt == NT - 1),
                    )

            def attn_unit(h, c):
                # one head, one 512-wide i-chunk. Score blocks run one jb
                # ahead of the attn@v accumulation so the PE has independent
                # work while ACT computes each exp batch.
                ps_out = ps_o.tile([128, 512], F32, tag="po")
                out_ap = ps_out[0:65, :]
                ets = [scores_jb(h, c, 0), scores_jb(h, c, 1)]
                av_jb(h, out_ap, 0, ets[0])
                ets.append(scores_jb(h, c, 2))
                av_jb(h, out_ap, 1, ets[1])
                ets.append(scores_jb(h, c, 3))
                av_jb(h, out_ap, 2, ets[2])
                av_jb(h, out_ap, 3, ets[3])
                attn_tail(h, c, ps_out)

            def merge(ft, c):
                sl = slice(c * 512, (c + 1) * 512)
                nc.vector.tensor_add(yT[:, ft, sl], yT[:, ft, sl], yE[:, ft, sl])

            def outproj(nt):
                ps = ps_mm.tile([128, 512], F32, tag="mm")
                for ft in range(FT):
                    nc.tensor.matmul(
                        ps,
                        yT[:, ft, nt * 128:(nt + 1) * 128],
                        wout[:, ft, :],
                        start=(ft == 0),
                        stop=(ft == FT - 1),
                    )
                ot = outs.tile([128, DIM], F32, tag="ot")
                nc.vector.tensor_add(ot, ps, bout_bc)
                nc.sync.dma_start(out=out_d[nt * 128:(nt + 1) * 128, :], in_=ot)

            # ---- emission schedule -------------------------------------
            emit_qk(0)        # q heads 0,1
            emit_qk(4)        # k heads 0,1

            # unit (h=0, c=0) emits its score blocks interleaved with emit_v
            # halves so ACT's 64us exp stream starts ~12us earlier; its
            # attn@v runs after emit_v (vaug must precede it in PE order).
            u0_et = [scores_jb(0, 0, 0), scores_jb(0, 0, 1)]
            emit_v(0, NT // 2)
            u0_et += [scores_jb(0, 0, 2), scores_jb(0, 0, 3)]
            emit_v(NT // 2, NT)
            u0_po = ps_o.tile([128, 512], F32, tag="po")
            for jb in range(4):
                av_jb(0, u0_po[0:65, :], jb, u0_et[jb])
            attn_tail(0, 0, u0_po)
            emit_qk(1)        # q heads 2,3 (unit h=2 needs it)

            # chunk 0: remaining 7 units, weaving the other q/k tiles
            # (PE filler while ACT chews exp) and the chunk-0 A_hat units.
            rest_qk = [5, 2, 6, 3, 7]
            for h in range(1, H):
                attn_unit(h, 0)
                if h - 1 < len(rest_qk):
                    emit_qk(rest_qk[h - 1])
                if h >= 4:
                    ahat_unit(h - 4, 0)
                    merge(h - 4, 0)

            # chunk 1: weave chunk-0 out-projections + chunk-1 A_hat units.
            for h in range(H):
                attn_unit(h, 1)
                if h < 4:
                    outproj(h)
                else:
                    ahat_unit(h - 4, 1)
                    merge(h - 4, 1)
            for nt in range(4, NT):
                outproj(nt)

    nc.compile()
    return nc


def _get_program():
    global _PROGRAM
    if _PROGRAM is None:
        _PROGRAM = _build_program()
    return _PROGRAM


def kernel(x, adj, w_qkv, w_out, b_out):
    x = np.asarray(x, dtype=np.float32)
    adj = np.asarray(adj, dtype=np.float32)
    w_qkv = np.asarray(w_qkv, dtype=np.float32)
    w_out = np.asarray(w_out, dtype=np.float32)
    b_out = np.asarray(b_out, dtype=np.float32).reshape(1, DIM)

    # host-side: normalized adjacency bias, replicated (cheap: one 1024^2 pass)
    A = np.ceil(adj) + np.eye(N, dtype=np.float32)
    dinv = A.sum(axis=1) ** -0.5
    A_hat = (A * dinv[:, None]) * dinv[None, :]

    bf = ml_dtypes.bfloat16
    ahatT = np.ascontiguousarray(A_hat.T).astype(bf)
    wqkv_b = np.ascontiguousarray(w_qkv).astype(bf)
    wout_b = np.ascontiguousarray(w_out).astype(bf)

    nc = _get_program()
    in_maps = []
    for b in range(B):
        in_maps.append({
            "xT": np.ascontiguousarray(x[b].T).astype(bf),
            "wqkv": wqkv_b,
            "ahatT": ahatT,
            "wout": wout_b,
            "bout": b_out,
        })
    global _last_in_maps
    _last_in_maps = in_maps
    res = run_bass_kernel_spmd(nc, in_maps, list(range(B)))
    out = np.stack([res.results[b]["out"] for b in range(B)], axis=0)
    return out.astype(np.float32)


if __name__ == "__main__":
    rng = np.random.default_rng(0)
    x = rng.standard_normal((B, N, DIM), dtype=np.float32)
    adj = (rng.random((N, N), dtype=np.float32) < 0.05).astype(np.float32) * 0.5
    w_qkv = rng.standard_normal((DIM, 3 * F), dtype=np.float32) * DIM ** -0.5
    w_out = rng.standard_normal((F, DIM), dtype=np.float32) * F ** -0.5
    b_out = np.zeros(DIM, dtype=np.float32)
    out = kernel(x=x, adj=adj, w_qkv=w_qkv, w_out=w_out, b_out=b_out)
    print("out", out.shape, out.dtype, np.abs(out).max())
